# revision 63
# baseline (speedup 1.0000x reference)
"""Trainium2 Bass kernel for the smoothed Preisach hysteresis model.

Math: the reference per-step update
    s' = where(h_t > h_{t-1}, s + (1-s)*sigmoid((h_t-alpha)/temp),
                              s + (-1-s)*sigmoid((beta-h_t)/temp))
is a first-order affine recurrence. With u = (s+1)/2, the up-mask M_t
(1 if h rose else 0) and z = M - u:

    z' = (z + dM_t) * a_t,   a_t = sigmoid(-arg_t),
    arg[n,t] = p_t + alpha_n*q_t + beta_n*r_t   (host-known p,q,r)

dM = M_t - M_{t-1} is a host constant row, so the scan consumes one
ScalarE sigmoid stream and one constant stream.

The scan runs on the DVE via a custom uOp program (registered into the
per-NEFF DVE table at compile time).  The stock tensor_tensor_scan
routes its recurrence backward through the 8-stage datapath and runs at
~2.1 ns/col; the custom program interleaves TWO streams per partition so
the feedback distance (NEXT_ALU_OUT_A, 2 cycles) lands exactly on the
same stream -> 1 elem/cycle, and a 2X_1PORT packed-fp16 variant runs
FOUR streams at 2 elem/cycle (~0.57 ns/col measured).

Sharding: hysteron dim N=5151 split as 8 cores x 640 rows + 31 rows on
the host (0.6%).  Per core: a packed tile of 512 rows laid out as
[128 partitions, 4 streams x T cols] scanned at 2x, plus a stock
[128, T] tile for the remaining 128 rows.  This keeps every engine at
the minimum column count (20480/core): ACT sigmoid ~17us, PE (arg +
readout matmuls) ~18us, DVE ~18us.

The packed tile is chunked (512 t-steps) through a "mega" buffer whose
per-chunk segments are [head(4) dm-body(2048) pad0(4) echo(4)
zbody(2048)]: the scan reads [head dm-body pad0] and writes [echo zbody
head_next], so its 4 pass-through tail outputs seed the next chunk's
recurrence with no separate carry instruction.  The interleaved arg
layout is produced by a K=24 block-sparse rhs (v24) so every PE psum
write is contiguous (strided psum writes measured ~4x slow), and the
readout is software-pipelined 2 chunks behind the scans so the in-order
PE/ACT queues never stall on a scan.

Readout: m_t = sum_n d_n z_nt via PSUM-accumulated K=128 matmuls; the
packed tile uses four stride-4 rhs access patterns (one per stream).
Host applies the affine fixup  sum d*s = d16sum*(2M-1) - 2*sum d*z  and
the output epilogue.
"""

import sys

import numpy as np

sys.path.insert(0, "/opt/trn_rl_repo")

N = 5151
T = 4096
TEMP = 0.01
NCORES = 8
P = 128
NPC = 640                 # rows per core on device
NHOST = N - NCORES * NPC  # 31 rows handled on host
PKROWS = 512              # packed-2x rows per core (128 partitions x 4)
STROWS = 128              # stock-scan rows per core
TS = 512                  # t-steps per pipeline step (psum-bank granule)
NCH = T // TS             # 8 steps
PKC = 4 * TS              # packed body cols per scan chunk (2048)
# mega-layout per chunk: h(4) dmb(2048) pad0(4) echo(4) zbody(2048)
# the scan reads [h dmb pad0] and writes [echo zbody h_next]: its last 4
# outputs ARE the next chunk's carry head -- no carry instruction, no race
SEG = 4 + PKC + 4 + 4 + PKC   # 4108
SCIN = 4 + PKC + 4            # scan stream length (2056)
ZB = 4 + PKC + 4 + 4          # zbody offset within segment (2060)
ACHUNK = 4 + PKC + 4          # a-tile cols per chunk (2056)
K6 = 6

_PROG_CACHE = {}

# --------------------------------------------------------------------------
# Custom DVE op: interleaved affine scan z' = (z + dm) * a.
# Streams interleave along the free dim: stream j = c mod S, t = c div S,
#   z[j,t] = (z[j,t-1] + dm[p,c]) * a[p,c],  z[j,-1] = 0.
# REGULAR program: S=2 (1 elem/cyc).  2X_1PORT program: S=4 (2 elem/cyc).
# Two bubble uOps zero the feedback a-flops before any element is consumed;
# per-chunk carries enter as head columns with a=1, dm=carry.
# --------------------------------------------------------------------------

OP_NAME = "HYST_AFFINE_SCAN_ANT"


def _build_hyst_uops():
    from concourse.dve_uop import (
        AluInp,
        AluOp,
        DelayInp,
        InpSel,
        OutPath,
        OutSel,
        Trigger,
        UopConfig,
    )

    EN = 1

    def steady_1x():
        u = UopConfig()
        u.enable_input(InpSel.SRC_0, 1)   # a   -> PREV_DELAY_0 at blk0
        u.enable_input(InpSel.SRC_1, 2)   # dm  -> PREV_DELAY_1 at blk0
        u.enable_input(InpSel.ZERO, 3)
        dp = u.datapath_config
        dp[0].pass_through_delay(0, 1)
        dp[1].enable_alu(AluOp.ADD, AluInp.NEXT_ALU_OUT_A, AluInp.PREV_DELAY_1)
        dp[1].pass_through_delay(0)
        dp[2].enable_alu(AluOp.MULTIPLY, AluInp.PREV_ALU_OUT,
                         AluInp.PREV_DELAY_0)
        dp[2].alu_out_a_enable = EN
        for k in range(3, 8):
            dp[k].pass_through_alu()
        u.enable_output(OutSel.ALU_OUT, OutPath.WR0_LO)
        u.require_inp0 = EN
        u.require_inp1 = EN
        u.trigger = (Trigger.SRC_TENSOR_DONE, Trigger.NONE, Trigger.NONE)
        return u

    def bubble_1x(nxt):
        u = UopConfig()
        u.enable_input(InpSel.SRC_0, 1)
        u.enable_input(InpSel.SRC_1, 2)
        u.enable_input(InpSel.ZERO, 3)    # -> PREV_DELAY_2
        dp = u.datapath_config
        dp[0].pass_through_delay(2)
        dp[1].pass_through_delay(2)
        dp[2].enable_alu(AluOp.BYPASS, AluInp.PREV_DELAY_2,
                         AluInp.PREV_DELAY_2)
        dp[2].alu_out_a_enable = EN
        u.repeat_count = 1
        u.trigger = (Trigger.COUNT, Trigger.NONE, Trigger.NONE)
        u.next_uop = (nxt, 0, 0)
        return u

    def steady_2x():
        u = UopConfig()
        u.enable_input(InpSel.SRC_0, 1)     # a_lo  -> PD0
        u.enable_input(InpSel.SRC_1, 2)     # dm_lo -> PD1
        u.enable_input(InpSel.SRC_0_HI, 3)  # a_hi  -> PD2
        u.enable_input(InpSel.SRC_1_HI, 4)  # dm_hi -> PD3
        u.enable_input(InpSel.ZERO, 5)
        dp = u.datapath_config
        dp[0].pass_through_delay(0, 1, 2, 3)
        dp[1].enable_alu(AluOp.ADD, AluInp.NEXT_ALU_OUT_A, AluInp.PREV_DELAY_1)
        dp[1].pass_through_delay(0, 2, 3)
        dp[2].enable_alu(AluOp.MULTIPLY, AluInp.PREV_ALU_OUT,
                         AluInp.PREV_DELAY_0)
        dp[2].alu_out_a_enable = EN
        dp[2].pass_through_delay(2, 3)
        dp[3].enable_delay_from_src(DelayInp.PREV_ALU_OUT, 0)  # capture z_lo
        dp[3].pass_through_delay(2, 3)
        dp[4].enable_alu(AluOp.ADD, AluInp.NEXT_ALU_OUT_A, AluInp.PREV_DELAY_3)
        dp[4].pass_through_delay(0, 2)
        dp[5].enable_alu(AluOp.MULTIPLY, AluInp.PREV_ALU_OUT,
                         AluInp.PREV_DELAY_2)
        dp[5].alu_out_a_enable = EN
        dp[5].pass_through_delay(0)
        dp[6].pass_through_alu()
        dp[6].pass_through_delay(0)
        dp[7].pass_through_alu()
        dp[7].pass_through_delay(0)
        u.enable_output(OutSel.DELAY_0, OutPath.WR0_LO)
        u.enable_output(OutSel.ALU_OUT, OutPath.WR0_HI)
        u.require_inp0 = EN
        u.require_inp1 = EN
        u.trigger = (Trigger.SRC_TENSOR_DONE, Trigger.NONE, Trigger.NONE)
        return u

    def bubble_2x(nxt):
        u = UopConfig()
        u.enable_input(InpSel.SRC_0, 1)
        u.enable_input(InpSel.SRC_1, 2)
        u.enable_input(InpSel.SRC_0_HI, 3)
        u.enable_input(InpSel.SRC_1_HI, 4)
        u.enable_input(InpSel.ZERO, 5)      # -> PD4
        dp = u.datapath_config
        for k in range(4):
            dp[k].pass_through_delay(4)
        dp[2].enable_alu(AluOp.BYPASS, AluInp.PREV_DELAY_4,
                         AluInp.PREV_DELAY_4)
        dp[2].alu_out_a_enable = EN
        dp[4].pass_through_delay(4)
        dp[5].enable_alu(AluOp.BYPASS, AluInp.PREV_DELAY_4,
                         AluInp.PREV_DELAY_4)
        dp[5].alu_out_a_enable = EN
        u.repeat_count = 1
        u.trigger = (Trigger.COUNT, Trigger.NONE, Trigger.NONE)
        u.next_uop = (nxt, 0, 0)
        return u

    return ([bubble_1x(1), bubble_1x(2), steady_1x()],
            [bubble_2x(1), bubble_2x(2), steady_2x()])


def _hyst_reference(in0, in1, n_streams=4):
    a = np.asarray(in0, np.float32)
    dm = np.asarray(in1, np.float32).reshape(a.shape)
    out = np.empty_like(a)
    S = n_streams
    z = np.zeros(a.shape[:-1] + (S,), np.float32)
    for c in range(a.shape[-1]):
        j = c % S
        z[..., j] = (z[..., j] + dm[..., c]) * a[..., c]
        out[..., c] = z[..., j]
    return out


class _HystOp:
    """DveOp-alike carrying a hand-written uOp program."""

    name = OP_NAME
    subdim = False
    uops_sha = {}
    perf_en = {}

    def __init__(self):
        from concourse.dve_spec import Spec, Src0, Src1

        self._cache = {}
        self.spec = Spec(body=Src0 * Src1,
                         reference=lambda in0, in1: _hyst_reference(in0, in1))

    def compile(self, ver):
        if ver in self._cache:
            return self._cache[ver]
        from concourse import dve_ops as _dve_ops
        from concourse.dve_uop import DveOpSpec

        uops_1x, uops_2x = _build_hyst_uops()
        spec = DveOpSpec(
            name=self.name,
            opcode=_dve_ops.get_dve_sub_opcode(self.name),
            uops=uops_1x,
            uops_2x=uops_2x,
            perf_max=1,
            rd1_en=True,
        )
        spec.validate(ver)
        self._cache[ver] = spec
        return spec


_HYST_OP = None


def _register_hyst_op():
    global _HYST_OP
    from concourse import dve_ops as _dve_ops

    if _HYST_OP is not None:
        return _HYST_OP
    for op in _dve_ops.OPS:
        if op.name == OP_NAME:
            _HYST_OP = op
            return op
    _HYST_OP = _HystOp()
    _dve_ops.OPS.append(_HYST_OP)
    _dve_ops._SUB_OPCODE_FOR_NAME[OP_NAME] = (
        _dve_ops._CUSTOM_DVE_ROW_BASE + len(_dve_ops.OPS) - 1
    )
    _dve_ops.CUSTOM_DVE_SPECS[OP_NAME] = _HYST_OP.spec
    return _HYST_OP


def _emit_scan(nc, *, out, a, dm, perf_max=1):
    """Emit the interleaved affine scan (STT struct, 2 tensor streams)."""
    from concourse import bass_isa, mybir
    from concourse import dve_ops as _dve_ops
    from concourse.mybir import ImmediateValue

    op = _register_hyst_op()
    v = nc.vector
    bass_obj = v.bass
    if OP_NAME not in bass_obj.m.ant_custom_dve_ops:
        bass_obj.m.ant_custom_dve_ops = sorted(
            {*bass_obj.m.ant_custom_dve_ops, OP_NAME}
        )
    isa_opcode = bass_obj.isa.Opcode[
        f"NEURON_ISA_TPB_OPCODE_CUSTOM_DVE_ANT_"
        f"{bass_isa.CustomDveShape.STT.slot()}"
    ].value
    ins = [
        v.lower_ap(a, for_isa=True, opt=True),
        v.lower_ap(dm, for_isa=True, opt=True),
        ImmediateValue(dtype=mybir.dt.float32, value=0.0),
        ImmediateValue(dtype=mybir.dt.float32, value=0.0),
    ]
    outs = [v.lower_ap(out, for_isa=True, opt=True)]
    return v.add_instruction(
        bass_isa.InstCustomDveAnt(
            name=bass_obj.get_next_instruction_name(),
            op_name=OP_NAME,
            rd1_en=True,
            subdim=0,
            imm2=0.0,
            shape=bass_isa.CustomDveShape.STT,
            row=_dve_ops.get_dve_sub_opcode(OP_NAME),
            isa_opcode=isa_opcode,
            ins=ins,
            outs=outs,
            perf_max=perf_max,
        )
    )


# --------------------------------------------------------------------------
# Program
# --------------------------------------------------------------------------


def _build_program(reps=1, loop_n=0, skip=(), dump=False):
    import contextlib

    import concourse.bass as bass
    import concourse.tile as tile
    from concourse import bacc, mybir

    _register_hyst_op()

    f32 = mybir.dt.float32
    f16 = mybir.dt.float16
    nc = bacc.Bacc("TRN2", target_bir_lowering=False, debug=False,
                   num_devices=NCORES)

    wtpk_d = nc.dram_tensor("wtpk", [4 * K6, P], f16, kind="ExternalInput")
    wtst_d = nc.dram_tensor("wtst", [K6, P], f16, kind="ExternalInput")
    v6_d = nc.dram_tensor("v6", [K6, T], f16, kind="ExternalInput")
    v24_d = nc.dram_tensor("v24", [4 * K6, 4 * T], f16, kind="ExternalInput")
    dmpk_d = nc.dram_tensor("dmpk", [NCH * SEG + 4], f16,
                            kind="ExternalInput")
    dmst_d = nc.dram_tensor("dmst", [T], f16, kind="ExternalInput")
    ones4_d = nc.dram_tensor("ones4", [8], f16, kind="ExternalInput")
    dens4_d = nc.dram_tensor("dens4", [P, 4], f16, kind="ExternalInput")
    densst_d = nc.dram_tensor("densst", [P, 1], f16, kind="ExternalInput")
    m_d = nc.dram_tensor("m", [1, T], f32, kind="ExternalOutput")
    if dump:
        spk_d = nc.dram_tensor("spk", [P, NCH * SEG + 4], f16,
                               kind="ExternalOutput")
        sst_d = nc.dram_tensor("sst", [P, T], f16, kind="ExternalOutput")
        apk_d = nc.dram_tensor("apk", [P, NCH * ACHUNK], f16,
                               kind="ExternalOutput")

    wtpk_ap = wtpk_d.ap()
    wtst_ap = wtst_d.ap()
    v6_ap = v6_d.ap()
    v24_ap = v24_d.ap()
    dmpk_ap = dmpk_d.ap()
    dmst_ap = dmst_d.ap()
    ones4_ap = ones4_d.ap()
    dens4_ap = dens4_d.ap()
    densst_ap = densst_d.ap()
    m_ap = m_d.ap()

    ts = bass.ts
    Sigmoid = mybir.ActivationFunctionType.Sigmoid
    mult = mybir.AluOpType.mult
    add = mybir.AluOpType.add

    def strided(ap, col0, stride, count):
        """[P, count] view of a [P, cols] SBUF AP with free-dim stride."""
        part = ap.ap[0]
        return bass.AP(tensor=ap.tensor, offset=ap.offset + col0,
                       ap=[list(part), [stride, count]])

    with tile.TileContext(nc) as tc:
        from contextlib import ExitStack
        with ExitStack() as ctx:
            consts = ctx.enter_context(tc.tile_pool(name="consts", bufs=1))
            ast_pool = ctx.enter_context(tc.tile_pool(name="ast", bufs=2))
            ps_a24 = ctx.enter_context(
                tc.tile_pool(name="ps_a24", bufs=2, space="PSUM"))
            ps_a6 = ctx.enter_context(
                tc.tile_pool(name="ps_a6", bufs=2, space="PSUM"))
            ps_m = ctx.enter_context(
                tc.tile_pool(name="ps_m", bufs=2, space="PSUM"))

            wtpk_sb = consts.tile([4 * K6, P], f16)
            wtst_sb = consts.tile([K6, P], f16)
            v6_sb = consts.tile([K6, T], f16)
            v24_sb = consts.tile([4 * K6, 4 * T], f16)
            dens4_sb = consts.tile([P, 4], f16)
            densst_sb = consts.tile([P, 1], f16)
            mega_sb = consts.tile([P, NCH * SEG + 4], f16)
            dmst_sb = consts.tile([P, T], f16)
            apk_sb = consts.tile([P, NCH * ACHUNK], f16)
            s_st = consts.tile([P, T], f16)
            m_sb = consts.tile([1, T], f32)

            nc.sync.dma_start(out=wtpk_sb[:], in_=wtpk_ap[:, :])
            nc.sync.dma_start(out=wtst_sb[:], in_=wtst_ap[:, :])
            nc.sync.dma_start(out=v6_sb[:], in_=v6_ap[:, :])
            v24_ch = 4 * T // NCH
            for c in range(NCH):
                nc.sync.dma_start(
                    out=v24_sb[:, c * v24_ch:(c + 1) * v24_ch],
                    in_=v24_ap[:, c * v24_ch:(c + 1) * v24_ch])
            nc.sync.dma_start(out=dens4_sb[:], in_=dens4_ap[:, :])
            nc.sync.dma_start(out=densst_sb[:], in_=densst_ap[:, :])
            # partition-broadcast DMAs, chunked across queues
            for c in range(NCH):
                # [h dmb pad0] region of each segment (echo/zbody are outputs)
                src = bass.AP(tensor=dmpk_ap.tensor,
                              offset=dmpk_ap.offset + c * SEG,
                              ap=[[0, P], [1, SCIN]])
                nc.sync.dma_start(
                    out=mega_sb[:, c * SEG:c * SEG + SCIN], in_=src)
                src = bass.AP(tensor=dmst_ap.tensor,
                              offset=dmst_ap.offset + c * TS,
                              ap=[[0, P], [1, TS]])
                nc.sync.dma_start(out=dmst_sb[:, ts(c, TS)], in_=src)
            # a-tile heads/tails = 1.0: boundary pairs are contiguous 8 cols
            ones_src = lambda n: bass.AP(
                tensor=ones4_ap.tensor, offset=ones4_ap.offset,
                ap=[[0, P], [1, n]])
            nc.sync.dma_start(out=apk_sb[:, 0:4], in_=ones_src(4))
            for c in range(NCH - 1):
                lo = c * ACHUNK + 4 + PKC
                nc.sync.dma_start(out=apk_sb[:, lo:lo + 8], in_=ones_src(8))
            lo = (NCH - 1) * ACHUNK + 4 + PKC
            nc.sync.dma_start(out=apk_sb[:, lo:lo + 4], in_=ones_src(4))

            if loop_n:
                loop_cm = tc.For_i(
                    0, loop_n, 1,
                    hint_engines=(mybir.EngineType.PE,
                                  mybir.EngineType.Activation,
                                  mybir.EngineType.DVE,
                                  mybir.EngineType.Pool))
            else:
                loop_cm = contextlib.nullcontext()
            with loop_cm:
              for _rep in range(reps):
                LAG = 2
                state = {"mp": None}

                def readout(cc):
                    base = cc * SEG + ZB
                    mp = ps_m.tile([1, TS], f32, tag="m", name="mp")
                    spk_ap = mega_sb[:]
                    for j in range(4):
                        if "contig" in skip:  # timing probe: wrong math
                            rhs = strided(spk_ap, base + j * TS, 1, TS)
                        else:
                            rhs = strided(spk_ap, base + j, 4, TS)
                        nc.tensor.matmul(
                            out=mp[:],
                            lhsT=dens4_sb[:, j:j + 1],
                            rhs=rhs,
                            start=(j == 0), stop=False,
                        )
                    nc.tensor.matmul(out=mp[:], lhsT=densst_sb[:, 0:1],
                                     rhs=s_st[:, ts(cc, TS)],
                                     start=False, stop=True)
                    # alternate the psum->sbuf drain between ACT and DVE so
                    # neither queue eats all 8 copies
                    if cc % 2 == 0:
                        nc.scalar.copy(out=m_sb[:, ts(cc, TS)], in_=mp[:])
                    else:
                        nc.vector.tensor_copy(out=m_sb[:, ts(cc, TS)],
                                              in_=mp[:])
                    nc.sync.dma_start(out=m_ap[:, ts(cc, TS)],
                                      in_=m_sb[:, ts(cc, TS)])

                for c in range(NCH):
                    base = c * ACHUNK + 4
                    # --- stock arg + sigmoid first: its scan only needs one
                    # ACT batch, so DVE can run it while ACT produces the
                    # two packed batches (overlaps the two serial chains and
                    # spaces consecutive packed scans apart) ---
                    argst = ps_a6.tile([P, TS], f32, tag="a6")
                    nc.tensor.matmul(out=argst[:], lhsT=wtst_sb[:],
                                     rhs=v6_sb[:, ts(c, TS)],
                                     start=True, stop=True)
                    ast = ast_pool.tile([P, TS], f16)
                    nc.scalar.activation(out=ast[:], in_=argst[:],
                                         func=Sigmoid, scale=-1.0)
                    if "scan" not in skip:
                        init = (0.0 if c == 0 else s_st[:, c * TS - 1:c * TS])
                        st = nc.vector.tensor_tensor_scan(
                            out=s_st[:, ts(c, TS)],
                            data0=dmst_sb[:, ts(c, TS)],
                            data1=ast[:],
                            initial=init, op0=add, op1=mult)
                        state["st"] = st
                    else:
                        nc.vector.tensor_copy(out=s_st[:, ts(c, TS)],
                                              in_=ast[:])
                    # --- packed arg matmuls + sigmoid (2 x 1024 cols) ---
                    # interleaving is encoded in v24's block-sparse columns,
                    # so every psum write is contiguous (strided psum writes
                    # measured ~4x slow)
                    for aj in range(2):
                        argp = ps_a24.tile([P, 1024], f32, tag="a24")
                        for b in range(2):
                            col0 = c * PKC + aj * 1024 + b * 512
                            nc.tensor.matmul(
                                out=argp[:, b * 512:(b + 1) * 512],
                                lhsT=wtpk_sb[:],
                                rhs=v24_sb[:, col0:col0 + 512],
                                start=True, stop=True,
                            )
                        nc.scalar.activation(
                            out=apk_sb[:, base + aj * 1024:
                                       base + aj * 1024 + 1024],
                            in_=argp[:], func=Sigmoid, scale=-1.0)
                    # --- packed scan: reads [h dmb pad0], writes [echo zbody
                    # h_next]; the last 4 outputs seed the next chunk ---
                    if "scan" not in skip:
                        pk = _emit_scan(
                            nc,
                            out=mega_sb[:, c * SEG + SCIN:
                                        c * SEG + SCIN + SCIN],
                            a=apk_sb[:, c * ACHUNK:(c + 1) * ACHUNK],
                            dm=mega_sb[:, c * SEG:c * SEG + SCIN],
                            perf_max=1)
                    else:
                        nc.vector.tensor_copy(
                            out=mega_sb[:, c * SEG + SCIN:
                                        c * SEG + 2 * SCIN],
                            in_=apk_sb[:, c * ACHUNK:(c + 1) * ACHUNK])
                    # --- lagged readout: PE/ACT queues never wait on the
                    # current chunk's scan ---
                    if "readout" in skip:
                        continue
                    if c >= LAG:
                        readout(c - LAG)
                if "readout" not in skip:
                    for cc in range(max(0, NCH - LAG), NCH):
                        readout(cc)
                if dump:
                    nc.sync.dma_start(out=spk_d.ap()[:, :], in_=mega_sb[:])
                    nc.sync.dma_start(out=sst_d.ap()[:, :], in_=s_st[:])
                    nc.sync.dma_start(out=apk_d.ap()[:, :], in_=apk_sb[:])
    nc.compile()
    return nc


# --------------------------------------------------------------------------
# Host side
# --------------------------------------------------------------------------


def _split16(x):
    hi = x.astype(np.float16)
    lo = (x - hi.astype(np.float64)).astype(np.float16)
    return hi, lo


def _host_prep(h, mesh_points, raw_density):
    h = np.asarray(h, np.float32)
    mesh = np.asarray(mesh_points, np.float32)
    rd = np.asarray(raw_density, np.float32)
    beta = mesh[:, 0].astype(np.float64)
    alpha = mesh[:, 1].astype(np.float64)

    hprev = np.concatenate([[np.float32(0.0)], h[:-1]])
    up = h > hprev
    R = np.float64(1.0) / np.float64(np.float32(TEMP))
    h64 = h.astype(np.float64)
    q = np.where(up, -R, 0.0)
    r = np.where(up, 0.0, R)
    p = np.where(up, R * h64, -R * h64)
    p_hi, p_lo = _split16(p)
    q16 = q.astype(np.float16)
    r16 = r.astype(np.float16)
    V6 = np.stack([q16, q16, r16, r16, p_hi, p_lo]).astype(np.float16)

    M = up.astype(np.float64)                 # M_t in {0,1}
    Mprev = np.concatenate([[0.0], M[:-1]])
    dM = (M - Mprev).astype(np.float16)       # in {-1,0,1}

    # packed-interleaved dM with zeroed 4-col chunk heads
    dmpk = np.zeros(NCH * SEG + 4, np.float16)
    dil = np.repeat(dM, 4)                    # [4T], col 4t+j
    for c in range(NCH):
        dmpk[c * SEG + 4:c * SEG + 4 + PKC] = dil[c * PKC:(c + 1) * PKC]

    dens = 1.0 / (1.0 + np.exp(-rd.astype(np.float64)))  # [N] float64

    # block-sparse interleaved basis: v24[(j,i), 4t+j'] = (j==j') * V6[i, t]
    v24 = np.zeros((4 * K6, 4 * T), np.float16)
    for j in range(4):
        v24[j * K6:(j + 1) * K6, j::4] = V6

    in_maps = []
    d16sum = 0.0
    for c in range(NCORES):
        rows = slice(c * NPC, (c + 1) * NPC)
        a_c = alpha[rows]
        b_c = beta[rows]
        d_c = dens[rows]
        # packed rows: row rr<512 -> partition rr//4, stream rr%4
        apk = a_c[:PKROWS].reshape(P, 4)
        bpk = b_c[:PKROWS].reshape(P, 4)
        dpk = d_c[:PKROWS].reshape(P, 4)
        wtpk = np.zeros((4 * K6, P), np.float16)
        for j in range(4):
            ah, al = _split16(apk[:, j])
            bh, bl = _split16(bpk[:, j])
            wtpk[j * K6 + 0] = ah
            wtpk[j * K6 + 1] = al
            wtpk[j * K6 + 2] = bh
            wtpk[j * K6 + 3] = bl
            wtpk[j * K6 + 4] = 1.0
            wtpk[j * K6 + 5] = 1.0
        ah, al = _split16(a_c[PKROWS:])
        bh, bl = _split16(b_c[PKROWS:])
        wtst = np.stack([ah, al, bh, bl,
                         np.ones(P, np.float16),
                         np.ones(P, np.float16)]).astype(np.float16)
        dens4 = dpk.astype(np.float16)
        densst = d_c[PKROWS:].astype(np.float16).reshape(P, 1)
        d16sum += dens4.astype(np.float64).sum()
        d16sum += densst.astype(np.float64).sum()
        in_maps.append({
            "wtpk": wtpk,
            "wtst": wtst,
            "v6": V6,
            "v24": v24,
            "dmpk": dmpk,
            "dmst": dM,
            "ones4": np.ones(8, np.float16),
            "dens4": dens4,
            "densst": densst,
        })

    host = {
        "alpha": alpha[NCORES * NPC:],
        "beta": beta[NCORES * NPC:],
        "dens": dens[NCORES * NPC:],
        "up": up,
        "h64": h64,
        "R": R,
        "dM": dM.astype(np.float64),
        "M": M,
    }
    return in_maps, dens, h, d16sum, host


def _host_rows_mz(host):
    """sum_n d_n z_nt for the NHOST residual rows, in float64."""
    alpha = host["alpha"]
    beta = host["beta"]
    d = host["dens"]
    up = host["up"]
    h64 = host["h64"]
    R = host["R"]
    dM = host["dM"]
    n = alpha.shape[0]
    if n == 0:
        return np.zeros(T)
    z = np.zeros(n)
    out = np.empty(T)
    qa = np.where(up[:, None], R * (h64[:, None] - alpha[None, :]),
                  R * (beta[None, :] - h64[:, None]))
    a = 1.0 / (1.0 + np.exp(qa))  # sigmoid(-arg)
    for t_ in range(T):
        z = (z + dM[t_]) * a[t_]
        out[t_] = d @ z
    return out


def kernel(h, mesh_points, raw_density, raw_offset, raw_scale, raw_slope):
    from concourse.bass_utils import run_bass_kernel_spmd

    in_maps, dens, h32, d16sum, host = _host_prep(h, mesh_points, raw_density)

    if "prog" not in _PROG_CACHE:
        _PROG_CACHE["prog"] = _build_program()
    nc = _PROG_CACHE["prog"]

    res = run_bass_kernel_spmd(nc, in_maps, list(range(NCORES)))
    zpart = np.zeros(T, np.float64)
    for c in range(NCORES):
        zpart += res.results[c]["m"].astype(np.float64).reshape(T)
    zpart += _host_rows_mz(host)

    def sigm(x):
        return 1.0 / (1.0 + np.exp(-np.float64(np.asarray(x, np.float32)[0])))

    offset = -10.0 + 20.0 * sigm(raw_offset)
    scale = 20.0 * sigm(raw_scale)
    slope = -20.0 + 40.0 * sigm(raw_slope)

    d16sum += host["dens"].sum()
    M = host["M"]
    # s = 2u-1, u = M - z  =>  sum(d*s) = d16sum*(2M-1) - 2*sum(d*z)
    m = (d16sum * (2.0 * M - 1.0) - 2.0 * zpart) / dens.sum()
    out = scale * m + h32.astype(np.float64) * slope + offset
    return out.astype(np.float32)


# revision 64
# speedup vs baseline: 1.2062x; 1.2062x over previous
"""Trainium2 Bass kernel for the smoothed Preisach hysteresis model.

Math: the reference per-step update
    s' = where(h_t > h_{t-1}, s + (1-s)*sigmoid((h_t-alpha)/temp),
                              s + (-1-s)*sigmoid((beta-h_t)/temp))
is a first-order affine recurrence. With u = (s+1)/2, the up-mask M_t
(1 if h rose else 0) and z = M - u:

    z' = (z + dM_t) * a_t,   a_t = sigmoid(-arg_t),
    arg[n,t] = p_t + alpha_n*q_t + beta_n*r_t   (host-known p,q,r)

dM = M_t - M_{t-1} is a host constant row, so the scan consumes one
ScalarE sigmoid stream and one constant stream.

The scan runs on the DVE via a custom uOp program (registered into the
per-NEFF DVE table at compile time).  The stock tensor_tensor_scan
routes its recurrence backward through the 8-stage datapath and runs at
~2.1 ns/col; the custom program interleaves TWO streams per partition so
the feedback distance (NEXT_ALU_OUT_A, 2 cycles) lands exactly on the
same stream -> 1 elem/cycle, and a 2X_1PORT packed-fp16 variant runs
FOUR streams at 2 elem/cycle (~0.57 ns/col measured).

Sharding: hysteron dim N=5151 split as 8 cores x 640 rows + 31 rows on
the host (0.6%).  Per core: a packed tile of 512 rows laid out as
[128 partitions, 4 streams x T cols] scanned at 2x, plus a stock
[128, T] tile for the remaining 128 rows.  This keeps every engine at
the minimum column count (20480/core): ACT sigmoid ~17us, PE (arg +
readout matmuls) ~18us, DVE ~18us.

The packed tile is chunked (512 t-steps) through a "mega" buffer whose
per-chunk segments are [head(4) dm-body(2048) pad0(4) echo(4)
zbody(2048)]: the scan reads [head dm-body pad0] and writes [echo zbody
head_next], so its 4 pass-through tail outputs seed the next chunk's
recurrence with no separate carry instruction.  The interleaved arg
layout is produced by a K=24 block-sparse rhs (v24) so every PE psum
write is contiguous (strided psum writes measured ~4x slow), and the
readout is software-pipelined 2 chunks behind the scans so the in-order
PE/ACT queues never stall on a scan.

Readout: m_t = sum_n d_n z_nt via PSUM-accumulated K=128 matmuls; the
packed tile uses four stride-4 rhs access patterns (one per stream).
Host applies the affine fixup  sum d*s = d16sum*(2M-1) - 2*sum d*z  and
the output epilogue.
"""

import sys

import numpy as np

sys.path.insert(0, "/opt/trn_rl_repo")

N = 5151
T = 4096
TEMP = 0.01
NCORES = 8
P = 128
NPC = 640                 # rows per core on device
NHOST = N - NCORES * NPC  # 31 rows handled on host
PKROWS = 512              # packed-2x rows per core (128 partitions x 4)
STROWS = 128              # stock-scan rows per core
TS = 512                  # t-steps per pipeline step (psum-bank granule)
NCH = T // TS             # 8 steps
PKC = 4 * TS              # packed body cols per scan chunk (2048)
# mega-layout per chunk: h(4) dmb(2048) pad0(4) echo(4) zbody(2048)
# the scan reads [h dmb pad0] and writes [echo zbody h_next]: its last 4
# outputs ARE the next chunk's carry head -- no carry instruction, no race
SEG = 4 + PKC + 4 + 4 + PKC   # 4108
SCIN = 4 + PKC + 4            # scan stream length (2056)
ZB = 4 + PKC + 4 + 4          # zbody offset within segment (2060)
ACHUNK = 4 + PKC + 4          # a-tile cols per chunk (2056)
K6 = 6

_PROG_CACHE = {}

# --------------------------------------------------------------------------
# Custom DVE op: interleaved affine scan z' = (z + dm) * a.
# Streams interleave along the free dim: stream j = c mod S, t = c div S,
#   z[j,t] = (z[j,t-1] + dm[p,c]) * a[p,c],  z[j,-1] = 0.
# REGULAR program: S=2 (1 elem/cyc).  2X_1PORT program: S=4 (2 elem/cyc).
# Two bubble uOps zero the feedback a-flops before any element is consumed;
# per-chunk carries enter as head columns with a=1, dm=carry.
# --------------------------------------------------------------------------

OP_NAME = "HYST_AFFINE_SCAN_ANT"


def _build_hyst_uops():
    from concourse.dve_uop import (
        AluInp,
        AluOp,
        DelayInp,
        InpSel,
        OutPath,
        OutSel,
        Trigger,
        UopConfig,
    )

    EN = 1

    def steady_1x():
        u = UopConfig()
        u.enable_input(InpSel.SRC_0, 1)   # a   -> PREV_DELAY_0 at blk0
        u.enable_input(InpSel.SRC_1, 2)   # dm  -> PREV_DELAY_1 at blk0
        u.enable_input(InpSel.ZERO, 3)
        dp = u.datapath_config
        dp[0].pass_through_delay(0, 1)
        dp[1].enable_alu(AluOp.ADD, AluInp.NEXT_ALU_OUT_A, AluInp.PREV_DELAY_1)
        dp[1].pass_through_delay(0)
        dp[2].enable_alu(AluOp.MULTIPLY, AluInp.PREV_ALU_OUT,
                         AluInp.PREV_DELAY_0)
        dp[2].alu_out_a_enable = EN
        for k in range(3, 8):
            dp[k].pass_through_alu()
        u.enable_output(OutSel.ALU_OUT, OutPath.WR0_LO)
        u.require_inp0 = EN
        u.require_inp1 = EN
        u.trigger = (Trigger.SRC_TENSOR_DONE, Trigger.NONE, Trigger.NONE)
        return u

    def bubble_1x(nxt):
        u = UopConfig()
        u.enable_input(InpSel.SRC_0, 1)
        u.enable_input(InpSel.SRC_1, 2)
        u.enable_input(InpSel.ZERO, 3)    # -> PREV_DELAY_2
        dp = u.datapath_config
        dp[0].pass_through_delay(2)
        dp[1].pass_through_delay(2)
        dp[2].enable_alu(AluOp.BYPASS, AluInp.PREV_DELAY_2,
                         AluInp.PREV_DELAY_2)
        dp[2].alu_out_a_enable = EN
        u.repeat_count = 1
        u.trigger = (Trigger.COUNT, Trigger.NONE, Trigger.NONE)
        u.next_uop = (nxt, 0, 0)
        return u

    def steady_2x():
        u = UopConfig()
        u.enable_input(InpSel.SRC_0, 1)     # a_lo  -> PD0
        u.enable_input(InpSel.SRC_1, 2)     # dm_lo -> PD1
        u.enable_input(InpSel.SRC_0_HI, 3)  # a_hi  -> PD2
        u.enable_input(InpSel.SRC_1_HI, 4)  # dm_hi -> PD3
        u.enable_input(InpSel.ZERO, 5)
        dp = u.datapath_config
        dp[0].pass_through_delay(0, 1, 2, 3)
        dp[1].enable_alu(AluOp.ADD, AluInp.NEXT_ALU_OUT_A, AluInp.PREV_DELAY_1)
        dp[1].pass_through_delay(0, 2, 3)
        dp[2].enable_alu(AluOp.MULTIPLY, AluInp.PREV_ALU_OUT,
                         AluInp.PREV_DELAY_0)
        dp[2].alu_out_a_enable = EN
        dp[2].pass_through_delay(2, 3)
        dp[3].enable_delay_from_src(DelayInp.PREV_ALU_OUT, 0)  # capture z_lo
        dp[3].pass_through_delay(2, 3)
        dp[4].enable_alu(AluOp.ADD, AluInp.NEXT_ALU_OUT_A, AluInp.PREV_DELAY_3)
        dp[4].pass_through_delay(0, 2)
        dp[5].enable_alu(AluOp.MULTIPLY, AluInp.PREV_ALU_OUT,
                         AluInp.PREV_DELAY_2)
        dp[5].alu_out_a_enable = EN
        dp[5].pass_through_delay(0)
        dp[6].pass_through_alu()
        dp[6].pass_through_delay(0)
        dp[7].pass_through_alu()
        dp[7].pass_through_delay(0)
        u.enable_output(OutSel.DELAY_0, OutPath.WR0_LO)
        u.enable_output(OutSel.ALU_OUT, OutPath.WR0_HI)
        u.require_inp0 = EN
        u.require_inp1 = EN
        u.trigger = (Trigger.SRC_TENSOR_DONE, Trigger.NONE, Trigger.NONE)
        return u

    def bubble_2x(nxt):
        u = UopConfig()
        u.enable_input(InpSel.SRC_0, 1)
        u.enable_input(InpSel.SRC_1, 2)
        u.enable_input(InpSel.SRC_0_HI, 3)
        u.enable_input(InpSel.SRC_1_HI, 4)
        u.enable_input(InpSel.ZERO, 5)      # -> PD4
        dp = u.datapath_config
        for k in range(4):
            dp[k].pass_through_delay(4)
        dp[2].enable_alu(AluOp.BYPASS, AluInp.PREV_DELAY_4,
                         AluInp.PREV_DELAY_4)
        dp[2].alu_out_a_enable = EN
        dp[4].pass_through_delay(4)
        dp[5].enable_alu(AluOp.BYPASS, AluInp.PREV_DELAY_4,
                         AluInp.PREV_DELAY_4)
        dp[5].alu_out_a_enable = EN
        u.repeat_count = 1
        u.trigger = (Trigger.COUNT, Trigger.NONE, Trigger.NONE)
        u.next_uop = (nxt, 0, 0)
        return u

    return ([bubble_1x(1), bubble_1x(2), steady_1x()],
            [bubble_2x(1), bubble_2x(2), steady_2x()])


def _hyst_reference(in0, in1, n_streams=4):
    a = np.asarray(in0, np.float32)
    dm = np.asarray(in1, np.float32).reshape(a.shape)
    out = np.empty_like(a)
    S = n_streams
    z = np.zeros(a.shape[:-1] + (S,), np.float32)
    for c in range(a.shape[-1]):
        j = c % S
        z[..., j] = (z[..., j] + dm[..., c]) * a[..., c]
        out[..., c] = z[..., j]
    return out


class _HystOp:
    """DveOp-alike carrying a hand-written uOp program."""

    name = OP_NAME
    subdim = False
    uops_sha = {}
    perf_en = {}

    def __init__(self):
        from concourse.dve_spec import Spec, Src0, Src1

        self._cache = {}
        self.spec = Spec(body=Src0 * Src1,
                         reference=lambda in0, in1: _hyst_reference(in0, in1))

    def compile(self, ver):
        if ver in self._cache:
            return self._cache[ver]
        from concourse import dve_ops as _dve_ops
        from concourse.dve_uop import DveOpSpec

        uops_1x, uops_2x = _build_hyst_uops()
        spec = DveOpSpec(
            name=self.name,
            opcode=_dve_ops.get_dve_sub_opcode(self.name),
            uops=uops_1x,
            uops_2x=uops_2x,
            perf_max=1,
            rd1_en=True,
        )
        spec.validate(ver)
        self._cache[ver] = spec
        return spec


_HYST_OP = None


def _register_hyst_op():
    global _HYST_OP
    from concourse import dve_ops as _dve_ops

    if _HYST_OP is not None:
        return _HYST_OP
    for op in _dve_ops.OPS:
        if op.name == OP_NAME:
            _HYST_OP = op
            return op
    _HYST_OP = _HystOp()
    _dve_ops.OPS.append(_HYST_OP)
    _dve_ops._SUB_OPCODE_FOR_NAME[OP_NAME] = (
        _dve_ops._CUSTOM_DVE_ROW_BASE + len(_dve_ops.OPS) - 1
    )
    _dve_ops.CUSTOM_DVE_SPECS[OP_NAME] = _HYST_OP.spec
    return _HYST_OP


def _emit_scan(nc, *, out, a, dm, perf_max=1):
    """Emit the interleaved affine scan (STT struct, 2 tensor streams)."""
    from concourse import bass_isa, mybir
    from concourse import dve_ops as _dve_ops
    from concourse.mybir import ImmediateValue

    op = _register_hyst_op()
    v = nc.vector
    bass_obj = v.bass
    if OP_NAME not in bass_obj.m.ant_custom_dve_ops:
        bass_obj.m.ant_custom_dve_ops = sorted(
            {*bass_obj.m.ant_custom_dve_ops, OP_NAME}
        )
    isa_opcode = bass_obj.isa.Opcode[
        f"NEURON_ISA_TPB_OPCODE_CUSTOM_DVE_ANT_"
        f"{bass_isa.CustomDveShape.STT.slot()}"
    ].value
    ins = [
        v.lower_ap(a, for_isa=True, opt=True),
        v.lower_ap(dm, for_isa=True, opt=True),
        ImmediateValue(dtype=mybir.dt.float32, value=0.0),
        ImmediateValue(dtype=mybir.dt.float32, value=0.0),
    ]
    outs = [v.lower_ap(out, for_isa=True, opt=True)]
    return v.add_instruction(
        bass_isa.InstCustomDveAnt(
            name=bass_obj.get_next_instruction_name(),
            op_name=OP_NAME,
            rd1_en=True,
            subdim=0,
            imm2=0.0,
            shape=bass_isa.CustomDveShape.STT,
            row=_dve_ops.get_dve_sub_opcode(OP_NAME),
            isa_opcode=isa_opcode,
            ins=ins,
            outs=outs,
            perf_max=perf_max,
        )
    )


# --------------------------------------------------------------------------
# Program
# --------------------------------------------------------------------------


def _build_program(reps=1, loop_n=0, skip=(), dump=False):
    import contextlib

    import concourse.bass as bass
    import concourse.tile as tile
    from concourse import bacc, mybir

    _register_hyst_op()

    f32 = mybir.dt.float32
    f16 = mybir.dt.float16
    nc = bacc.Bacc("TRN2", target_bir_lowering=False, debug=False,
                   num_devices=NCORES)

    wtpk_d = nc.dram_tensor("wtpk", [4 * K6, P], f16, kind="ExternalInput")
    wtst_d = nc.dram_tensor("wtst", [K6, P], f16, kind="ExternalInput")
    v6_d = nc.dram_tensor("v6", [K6, T], f16, kind="ExternalInput")
    v24_d = nc.dram_tensor("v24", [4 * K6, 4 * T], f16, kind="ExternalInput")
    dmpk_d = nc.dram_tensor("dmpk", [NCH * SEG + 4], f16,
                            kind="ExternalInput")
    dmst_d = nc.dram_tensor("dmst", [T], f16, kind="ExternalInput")
    ones4_d = nc.dram_tensor("ones4", [8], f16, kind="ExternalInput")
    dens4_d = nc.dram_tensor("dens4", [P, 4], f16, kind="ExternalInput")
    densst_d = nc.dram_tensor("densst", [P, 1], f16, kind="ExternalInput")
    m_d = nc.dram_tensor("m", [1, T], f32, kind="ExternalOutput")
    if dump:
        spk_d = nc.dram_tensor("spk", [P, NCH * SEG + 4], f16,
                               kind="ExternalOutput")
        sst_d = nc.dram_tensor("sst", [P, T], f16, kind="ExternalOutput")
        apk_d = nc.dram_tensor("apk", [P, NCH * ACHUNK], f16,
                               kind="ExternalOutput")

    wtpk_ap = wtpk_d.ap()
    wtst_ap = wtst_d.ap()
    v6_ap = v6_d.ap()
    v24_ap = v24_d.ap()
    dmpk_ap = dmpk_d.ap()
    dmst_ap = dmst_d.ap()
    ones4_ap = ones4_d.ap()
    dens4_ap = dens4_d.ap()
    densst_ap = densst_d.ap()
    m_ap = m_d.ap()

    ts = bass.ts
    Sigmoid = mybir.ActivationFunctionType.Sigmoid
    mult = mybir.AluOpType.mult
    add = mybir.AluOpType.add

    def strided(ap, col0, stride, count):
        """[P, count] view of a [P, cols] SBUF AP with free-dim stride."""
        part = ap.ap[0]
        return bass.AP(tensor=ap.tensor, offset=ap.offset + col0,
                       ap=[list(part), [stride, count]])

    with tile.TileContext(nc) as tc:
        from contextlib import ExitStack
        with ExitStack() as ctx:
            consts = ctx.enter_context(tc.tile_pool(name="consts", bufs=1))
            ast_pool = ctx.enter_context(tc.tile_pool(name="ast", bufs=2))
            ps_a24 = ctx.enter_context(
                tc.tile_pool(name="ps_a24", bufs=2, space="PSUM"))
            ps_a6 = ctx.enter_context(
                tc.tile_pool(name="ps_a6", bufs=2, space="PSUM"))
            ps_m = ctx.enter_context(
                tc.tile_pool(name="ps_m", bufs=2, space="PSUM"))

            wtpk_sb = consts.tile([4 * K6, P], f16)
            wtst_sb = consts.tile([K6, P], f16)
            v6_sb = consts.tile([K6, T], f16)
            v24_sb = consts.tile([4 * K6, 4 * T], f16)
            dens4_sb = consts.tile([P, 4], f16)
            densst_sb = consts.tile([P, 1], f16)
            mega_sb = consts.tile([P, NCH * SEG + 4], f16)
            dmst_sb = consts.tile([P, T], f16)
            apk_sb = consts.tile([P, NCH * ACHUNK], f16)
            s_st = consts.tile([P, T], f16)
            m_sb = consts.tile([1, T], f32)

            nc.sync.dma_start(out=wtpk_sb[:], in_=wtpk_ap[:, :])
            nc.sync.dma_start(out=wtst_sb[:], in_=wtst_ap[:, :])
            nc.sync.dma_start(out=v6_sb[:], in_=v6_ap[:, :])
            v24_ch = 4 * T // NCH
            for c in range(NCH):
                nc.sync.dma_start(
                    out=v24_sb[:, c * v24_ch:(c + 1) * v24_ch],
                    in_=v24_ap[:, c * v24_ch:(c + 1) * v24_ch])
            nc.sync.dma_start(out=dens4_sb[:], in_=dens4_ap[:, :])
            nc.sync.dma_start(out=densst_sb[:], in_=densst_ap[:, :])
            # partition-broadcast DMAs, chunked across queues
            for c in range(NCH):
                # [h dmb pad0] region of each segment (echo/zbody are outputs)
                src = bass.AP(tensor=dmpk_ap.tensor,
                              offset=dmpk_ap.offset + c * SEG,
                              ap=[[0, P], [1, SCIN]])
                nc.sync.dma_start(
                    out=mega_sb[:, c * SEG:c * SEG + SCIN], in_=src)
                src = bass.AP(tensor=dmst_ap.tensor,
                              offset=dmst_ap.offset + c * TS,
                              ap=[[0, P], [1, TS]])
                nc.sync.dma_start(out=dmst_sb[:, ts(c, TS)], in_=src)
            # a-tile heads/tails = 1.0: boundary pairs are contiguous 8 cols
            ones_src = lambda n: bass.AP(
                tensor=ones4_ap.tensor, offset=ones4_ap.offset,
                ap=[[0, P], [1, n]])
            nc.sync.dma_start(out=apk_sb[:, 0:4], in_=ones_src(4))
            for c in range(NCH - 1):
                lo = c * ACHUNK + 4 + PKC
                nc.sync.dma_start(out=apk_sb[:, lo:lo + 8], in_=ones_src(8))
            lo = (NCH - 1) * ACHUNK + 4 + PKC
            nc.sync.dma_start(out=apk_sb[:, lo:lo + 4], in_=ones_src(4))

            if loop_n:
                loop_cm = tc.For_i(
                    0, loop_n, 1,
                    hint_engines=(mybir.EngineType.PE,
                                  mybir.EngineType.Activation,
                                  mybir.EngineType.DVE,
                                  mybir.EngineType.Pool))
            else:
                loop_cm = contextlib.nullcontext()
            with loop_cm:
              for _rep in range(reps):
                LAG = 2
                state = {"mp": None}

                def readout(cc):
                    base = cc * SEG + ZB
                    mp = ps_m.tile([1, TS], f32, tag="m", name="mp")
                    spk_ap = mega_sb[:]
                    for j in range(4):
                        if "contig" in skip:  # timing probe: wrong math
                            rhs = strided(spk_ap, base + j * TS, 1, TS)
                        else:
                            rhs = strided(spk_ap, base + j, 4, TS)
                        nc.tensor.matmul(
                            out=mp[:],
                            lhsT=dens4_sb[:, j:j + 1],
                            rhs=rhs,
                            start=(j == 0), stop=False,
                        )
                    nc.tensor.matmul(out=mp[:], lhsT=densst_sb[:, 0:1],
                                     rhs=s_st[:, ts(cc, TS)],
                                     start=False, stop=True)
                    # alternate the psum->sbuf drain between ACT and DVE so
                    # neither queue eats all 8 copies
                    if cc % 2 == 0:
                        nc.scalar.copy(out=m_sb[:, ts(cc, TS)], in_=mp[:])
                    else:
                        nc.vector.tensor_copy(out=m_sb[:, ts(cc, TS)],
                                              in_=mp[:])
                    nc.sync.dma_start(out=m_ap[:, ts(cc, TS)],
                                      in_=m_sb[:, ts(cc, TS)])

                for c in range(NCH):
                    base = c * ACHUNK + 4
                    # --- packed arg matmuls + sigmoid (2 x 1024 cols) ---
                    # interleaving is encoded in v24's block-sparse columns,
                    # so every psum write is contiguous (strided psum writes
                    # measured ~4x slow)
                    for aj in range(2):
                        argp = ps_a24.tile([P, 1024], f32, tag="a24")
                        for b in range(2):
                            col0 = c * PKC + aj * 1024 + b * 512
                            nc.tensor.matmul(
                                out=argp[:, b * 512:(b + 1) * 512],
                                lhsT=wtpk_sb[:],
                                rhs=v24_sb[:, col0:col0 + 512],
                                start=True, stop=True,
                            )
                        nc.scalar.activation(
                            out=apk_sb[:, base + aj * 1024:
                                       base + aj * 1024 + 1024],
                            in_=argp[:], func=Sigmoid, scale=-1.0)
                    # --- stock arg + sigmoid (512 cols) ---
                    argst = ps_a6.tile([P, TS], f32, tag="a6")
                    nc.tensor.matmul(out=argst[:], lhsT=wtst_sb[:],
                                     rhs=v6_sb[:, ts(c, TS)],
                                     start=True, stop=True)
                    ast = ast_pool.tile([P, TS], f16)
                    nc.scalar.activation(out=ast[:], in_=argst[:],
                                         func=Sigmoid, scale=-1.0)
                    # --- scans: reads [h dmb pad0], writes [echo zbody
                    # h_next]; the last 4 outputs seed the next chunk ---
                    if "scan" not in skip:
                        pk = _emit_scan(
                            nc,
                            out=mega_sb[:, c * SEG + SCIN:
                                        c * SEG + SCIN + SCIN],
                            a=apk_sb[:, c * ACHUNK:(c + 1) * ACHUNK],
                            dm=mega_sb[:, c * SEG:c * SEG + SCIN],
                            perf_max=1)
                        init = (0.0 if c == 0 else s_st[:, c * TS - 1:c * TS])
                        st = nc.vector.tensor_tensor_scan(
                            out=s_st[:, ts(c, TS)],
                            data0=dmst_sb[:, ts(c, TS)],
                            data1=ast[:],
                            initial=init, op0=add, op1=mult)
                        state["st"] = st
                    else:
                        nc.vector.tensor_copy(
                            out=mega_sb[:, c * SEG + SCIN:
                                        c * SEG + 2 * SCIN],
                            in_=apk_sb[:, c * ACHUNK:(c + 1) * ACHUNK])
                        nc.vector.tensor_copy(out=s_st[:, ts(c, TS)],
                                              in_=ast[:])
                    # --- lagged readout: PE/ACT queues never wait on the
                    # current chunk's scan ---
                    if "readout" in skip:
                        continue
                    if c >= LAG:
                        readout(c - LAG)
                if "readout" not in skip:
                    for cc in range(max(0, NCH - LAG), NCH):
                        readout(cc)
                if dump:
                    nc.sync.dma_start(out=spk_d.ap()[:, :], in_=mega_sb[:])
                    nc.sync.dma_start(out=sst_d.ap()[:, :], in_=s_st[:])
                    nc.sync.dma_start(out=apk_d.ap()[:, :], in_=apk_sb[:])
    nc.compile()
    return nc


# --------------------------------------------------------------------------
# Host side
# --------------------------------------------------------------------------


def _split16(x):
    hi = x.astype(np.float16)
    lo = (x - hi.astype(np.float64)).astype(np.float16)
    return hi, lo


def _host_prep(h, mesh_points, raw_density):
    h = np.asarray(h, np.float32)
    mesh = np.asarray(mesh_points, np.float32)
    rd = np.asarray(raw_density, np.float32)
    beta = mesh[:, 0].astype(np.float64)
    alpha = mesh[:, 1].astype(np.float64)

    hprev = np.concatenate([[np.float32(0.0)], h[:-1]])
    up = h > hprev
    R = np.float64(1.0) / np.float64(np.float32(TEMP))
    h64 = h.astype(np.float64)
    q = np.where(up, -R, 0.0)
    r = np.where(up, 0.0, R)
    p = np.where(up, R * h64, -R * h64)
    p_hi, p_lo = _split16(p)
    q16 = q.astype(np.float16)
    r16 = r.astype(np.float16)
    V6 = np.stack([q16, q16, r16, r16, p_hi, p_lo]).astype(np.float16)

    M = up.astype(np.float64)                 # M_t in {0,1}
    Mprev = np.concatenate([[0.0], M[:-1]])
    dM = (M - Mprev).astype(np.float16)       # in {-1,0,1}

    # packed-interleaved dM with zeroed 4-col chunk heads
    dmpk = np.zeros(NCH * SEG + 4, np.float16)
    dil = np.repeat(dM, 4)                    # [4T], col 4t+j
    for c in range(NCH):
        dmpk[c * SEG + 4:c * SEG + 4 + PKC] = dil[c * PKC:(c + 1) * PKC]

    dens = 1.0 / (1.0 + np.exp(-rd.astype(np.float64)))  # [N] float64

    # block-sparse interleaved basis: v24[(j,i), 4t+j'] = (j==j') * V6[i, t]
    v24 = np.zeros((4 * K6, 4 * T), np.float16)
    for j in range(4):
        v24[j * K6:(j + 1) * K6, j::4] = V6

    in_maps = []
    d16sum = 0.0
    for c in range(NCORES):
        rows = slice(c * NPC, (c + 1) * NPC)
        a_c = alpha[rows]
        b_c = beta[rows]
        d_c = dens[rows]
        # packed rows: row rr<512 -> partition rr//4, stream rr%4
        apk = a_c[:PKROWS].reshape(P, 4)
        bpk = b_c[:PKROWS].reshape(P, 4)
        dpk = d_c[:PKROWS].reshape(P, 4)
        wtpk = np.zeros((4 * K6, P), np.float16)
        for j in range(4):
            ah, al = _split16(apk[:, j])
            bh, bl = _split16(bpk[:, j])
            wtpk[j * K6 + 0] = ah
            wtpk[j * K6 + 1] = al
            wtpk[j * K6 + 2] = bh
            wtpk[j * K6 + 3] = bl
            wtpk[j * K6 + 4] = 1.0
            wtpk[j * K6 + 5] = 1.0
        ah, al = _split16(a_c[PKROWS:])
        bh, bl = _split16(b_c[PKROWS:])
        wtst = np.stack([ah, al, bh, bl,
                         np.ones(P, np.float16),
                         np.ones(P, np.float16)]).astype(np.float16)
        dens4 = dpk.astype(np.float16)
        densst = d_c[PKROWS:].astype(np.float16).reshape(P, 1)
        d16sum += dens4.astype(np.float64).sum()
        d16sum += densst.astype(np.float64).sum()
        in_maps.append({
            "wtpk": wtpk,
            "wtst": wtst,
            "v6": V6,
            "v24": v24,
            "dmpk": dmpk,
            "dmst": dM,
            "ones4": np.ones(8, np.float16),
            "dens4": dens4,
            "densst": densst,
        })

    host = {
        "alpha": alpha[NCORES * NPC:],
        "beta": beta[NCORES * NPC:],
        "dens": dens[NCORES * NPC:],
        "up": up,
        "h64": h64,
        "R": R,
        "dM": dM.astype(np.float64),
        "M": M,
    }
    return in_maps, dens, h, d16sum, host


def _host_rows_mz(host):
    """sum_n d_n z_nt for the NHOST residual rows, in float64."""
    alpha = host["alpha"]
    beta = host["beta"]
    d = host["dens"]
    up = host["up"]
    h64 = host["h64"]
    R = host["R"]
    dM = host["dM"]
    n = alpha.shape[0]
    if n == 0:
        return np.zeros(T)
    z = np.zeros(n)
    out = np.empty(T)
    qa = np.where(up[:, None], R * (h64[:, None] - alpha[None, :]),
                  R * (beta[None, :] - h64[:, None]))
    a = 1.0 / (1.0 + np.exp(qa))  # sigmoid(-arg)
    for t_ in range(T):
        z = (z + dM[t_]) * a[t_]
        out[t_] = d @ z
    return out


def kernel(h, mesh_points, raw_density, raw_offset, raw_scale, raw_slope):
    from concourse.bass_utils import run_bass_kernel_spmd

    in_maps, dens, h32, d16sum, host = _host_prep(h, mesh_points, raw_density)

    if "prog" not in _PROG_CACHE:
        _PROG_CACHE["prog"] = _build_program()
    nc = _PROG_CACHE["prog"]

    res = run_bass_kernel_spmd(nc, in_maps, list(range(NCORES)))
    zpart = np.zeros(T, np.float64)
    for c in range(NCORES):
        zpart += res.results[c]["m"].astype(np.float64).reshape(T)
    zpart += _host_rows_mz(host)

    def sigm(x):
        return 1.0 / (1.0 + np.exp(-np.float64(np.asarray(x, np.float32)[0])))

    offset = -10.0 + 20.0 * sigm(raw_offset)
    scale = 20.0 * sigm(raw_scale)
    slope = -20.0 + 40.0 * sigm(raw_slope)

    d16sum += host["dens"].sum()
    M = host["M"]
    # s = 2u-1, u = M - z  =>  sum(d*s) = d16sum*(2M-1) - 2*sum(d*z)
    m = (d16sum * (2.0 * M - 1.0) - 2.0 * zpart) / dens.sum()
    out = scale * m + h32.astype(np.float64) * slope + offset
    return out.astype(np.float32)
